# revision 7
# baseline (speedup 1.0000x reference)
"""CrossCorrelLoss kernel for Trainium2 (8 NeuronCores, data-parallel batch).

Math: the reference normalizes x over dims (0,1) (global mean / unbiased std
per channel), computes per-batch gram matrices of the normalized data, means
over batch, gathers tril entries and compares against cross_correl_real.
Because the normalization stats are global, everything collapses to the raw
second-moment matrix of the flattened (B*T, N) data:
    G = X^T X,  S1 = column sums of X,  M = B*T
    mu = S1/M,  var = (diag(G) - M mu^2)/(M-1)
    Cbar[i,j] = (G[i,j]/M - mu_i mu_j) / (sd_i sd_j)
    loss = sum |Cbar[tril] - cross_correl_real| / 10

Kernel structure (raw Bass, no TileContext):

  * x is padded with a constant ones column on the host (N=321 -> 322): the
    last weight row of the third G block then computes S1 = 1^T X on the
    tensor engine for free — no vector-engine reductions at all;
  * rows are laid out p-major (partition p holds 64 consecutive rows), so the
    input streams as a few large SWDGE DMAs that cast f32->bf16 in flight
    (measured at the full ~300 GB/s practical per-core line rate) — no fp32
    copy in SBUF and no scalar-engine cast pass;
  * only the lower-triangular row blocks of G are computed (streams
    128/256/322 columns per 128-row batch instead of 3x321) since the loss
    only needs tril entries; the matmuls run in bf16 (~302 ns per batch,
    LDWEIGHTS fully overlapped), well under the DMA span;
  * thin tail DMA slices (4/3/1 rows) keep the pipeline drain short;
  * matmul completion is signaled with .then_inc on the LAST matmul — a
    separate sem_inc races the PSUM writeback (the PE sequencer runs ahead of
    the array drain) and crashes the worker on this setup.

Each core produces G[0:128,0:128], G[128:256,0:256], G_aug[256:322,0:322]
(augmented row 321 = column sums) plus nothing else; the host sums the 8
partials in float64 and runs the tiny 321x321 finalization. bf16 rounding
washes out to ~2e-5 relative on the final scalar.
"""

import contextlib

import numpy as np

import concourse.bass as bass
import concourse.mybir as mybir
from concourse.bass_utils import run_bass_kernel_spmd

B, T, N = 128, 512, 321
NP1 = N + 1                  # ones column appended
NCORES = 8
M_TOTAL = B * T
M_CORE = M_TOTAL // NCORES   # 8192 rows per core
P = 128                      # SBUF partitions
Q = M_CORE // P              # 64 rows per partition (p-major)

# q-rows per input DMA slice; thin tail slices shorten the pipeline drain
SLICES = [8, 8, 8, 8, 8, 8, 8, 4, 3, 1]
assert sum(SLICES) == Q

# (weight col start, weight width, streamed cols) per G row block
BLOCKS = [(0, 128, 128), (128, 128, 256), (256, 66, NP1)]
OC = 128 + 256 + NP1         # output words per partition

_NC = None


def _build_nc(reps=1):
    # reps>1 repeats the whole body in one NEFF (timing rig only: slope
    # between two reps values isolates steady-state program time from
    # per-exec overhead; every rep fully drains). The harness path uses
    # reps=1.
    f32 = mybir.dt.float32
    bf16 = mybir.dt.bfloat16

    nc = bass.Bass()
    x = nc.declare_dram_parameter("x", [M_CORE, NP1], f32, isOutput=False)
    o_out = nc.declare_dram_parameter("o", [P, OC], f32, isOutput=True)

    # partition p holds rows [Q*p, Q*(p+1)): per-partition contiguous stream
    xv = x.rearrange("(p q) n -> p (q n)", p=P, q=Q)

    nslice = len(SLICES)
    bounds = np.cumsum([0] + SLICES)

    with contextlib.ExitStack() as ctx:
        xb = ctx.enter_context(nc.sbuf_tensor("xb", [P, Q, NP1], bf16))
        out_t = ctx.enter_context(nc.sbuf_tensor("out_t", [P, OC], f32))
        psums = [
            ctx.enter_context(nc.psum_tensor(f"psum{b}", [P, ncol], f32))
            for b, (_, _, ncol) in enumerate(BLOCKS)
        ]
        # one sem per input DMA: a shared counting sem is unsound because each
        # DMA completes as 16 independent SDMA-engine slices that can
        # interleave across consecutive DMAs
        dma_sems = [
            ctx.enter_context(nc.semaphore(f"dma_sem{s}")) for s in range(nslice)
        ]
        odma_sem = ctx.enter_context(nc.semaphore("odma_sem"))
        act_sem = ctx.enter_context(nc.semaphore("act_sem"))
        init_sem = ctx.enter_context(nc.semaphore("init_sem"))
        pe_sem = ctx.enter_context(nc.semaphore("pe_sem"))
        dve_sem = ctx.enter_context(nc.semaphore("dve_sem"))
        block = ctx.enter_context(nc.Block())

        @block.gpsimd
        def _(ge):
            for rep in range(reps):
                if rep > 0:
                    # full drain: next rep's input DMAs overwrite xb only after
                    # the previous rep's output round-trip completed
                    ge.wait_ge(odma_sem, 16 * rep)
                for s in range(nslice):
                    q0, q1 = bounds[s], bounds[s + 1]
                    ge.dma_start(
                        xb[:, q0:q1, :], xv[:, q0 * NP1 : q1 * NP1]
                    ).then_inc(dma_sems[s], 16)

        @block.tensor
        def _(te):
            for rep in range(reps):
                for s in range(nslice):
                    te.wait_ge(dma_sems[s], 16 * (rep + 1))
                    for j in range(bounds[s], bounds[s + 1]):
                        for bi, (i0, w, ncol) in enumerate(BLOCKS):
                            mm = te.matmul(
                                psums[bi][:w, :],
                                xb[:, j, i0 : i0 + w],
                                xb[:, j, 0:ncol],
                                start=(j == 0),
                                stop=(j == Q - 1),
                            )
                mm.then_inc(pe_sem, 1)

        @block.scalar
        def _(se):
            # the otherwise-idle scalar engine drains psum0/1 in parallel with
            # the vector engine's psum2 copy; the single output DMA waits on
            # both (the earlier split-output variant regressed — one DMA, two
            # sems is the measured winner: -0.4 us same-session)
            for rep in range(reps):
                se.wait_ge(pe_sem, rep + 1)
                se.copy(out_t[:, 0:128], psums[0][:])
                se.copy(out_t[:, 128:384], psums[1][:]).then_inc(act_sem, 1)

        @block.vector
        def _(ve):
            # partitions 66:128 of the block-2 region are never written by the
            # psum2 copy; engines can only address partition starts 0/32/64/96,
            # so clear 64: and let the copy overwrite rows 64-65 each rep
            ve.memset(out_t[64:, 384:OC], 0.0).then_inc(init_sem, 1)
            ve.wait_ge(init_sem, 1)
            for rep in range(reps):
                ve.wait_ge(pe_sem, rep + 1)
                ve.tensor_copy(out_t[:66, 384:OC], psums[2][:66, :]).then_inc(
                    dve_sem, 1
                )

        @block.sync
        def _(sync):
            for rep in range(reps):
                sync.wait_ge(act_sem, rep + 1)
                sync.wait_ge(dve_sem, rep + 1)
                sync.dma_start(o_out[:], out_t[:]).then_inc(odma_sem, 16)
                sync.wait_ge(odma_sem, 16 * (rep + 1))

    return nc


def _get_nc():
    global _NC
    if _NC is None:
        _NC = _build_nc()
    return _NC


def _finalize(o_parts, cross_correl_real):
    G = np.zeros((NP1, NP1), np.float64)
    for o in o_parts:
        o = np.asarray(o, dtype=np.float64)
        G[0:128, 0:128] += o[:, 0:128]
        G[128:256, 0:256] += o[:, 128:384]
        G[256:NP1, 0:NP1] += o[:66, 384:OC]
    M = float(M_TOTAL)
    S1 = G[N, 0:N]
    mu = S1 / M
    var = (np.diag(G)[0:N] - M * mu * mu) / (M - 1.0)
    sd = np.sqrt(var)
    C = (G[0:N, 0:N] / M - np.outer(mu, mu)) / np.outer(sd, sd)
    i0, i1 = np.tril_indices(N)
    loss = np.abs(C[i0, i1] - cross_correl_real.astype(np.float64)).sum() / 10.0
    return np.float32(loss)


def _shard_inputs(x_fake):
    x = np.asarray(x_fake, dtype=np.float32).reshape(M_TOTAL, N)
    xp = np.empty((M_TOTAL, NP1), np.float32)
    xp[:, :N] = x
    xp[:, N] = 1.0
    return [
        {"x": np.ascontiguousarray(xp[i * M_CORE : (i + 1) * M_CORE])}
        for i in range(NCORES)
    ]


def kernel(x_fake, cross_correl_real):
    nc = _get_nc()
    in_maps = _shard_inputs(x_fake)
    res = run_bass_kernel_spmd(nc, in_maps, list(range(NCORES))).results
    return _finalize([r["o"] for r in res], np.asarray(cross_correl_real))
